# revision 1
# baseline (speedup 1.0000x reference)
"""Trainium2 Bass kernel for nn_BasicCNN (conv bank + LoRA-masked recurrent net).

Hybrid TP4 x DP2 sharding (communication-optimal under the SBUF budget):
 - Cores 0-3 handle batch 0:512, cores 4-7 batch 512:1024 (DP groups).
 - Within each group, W1 = (W + 2*(A@B)*mask + I) is column-sharded 4-way
   ([4096, 1024] bf16, SBUF-resident), built on device from the LoRA factors
   and an fp8-shipped mask; the +I fold implements the residual connection.
 - State kept transposed [state_dim, batch_half] so W tiles are the stationary
   matmul operand and no transposes are ever needed.
 - Per timestep each group AllGathers its half's state ([4096, 512] bf16) in
   two 256-column chunks so one chunk's gather overlaps the other's compute.
   Remote traffic is 3 MB/core/timestep vs 7 MB for 8-way TP.
 - Conv bank = one dense matmul vs a host-assembled [512, 3328] scatter of the
   conv kernels; t1 contracts only the sensory block; t4 computes only each
   core's O-block slice; output projection sharded over output columns.
"""
import sys

for _p in ("/opt/trn_rl_repo", "/root/.axon_site/_ro/trn_rl_repo"):
    if _p not in sys.path:
        sys.path.append(_p)

import numpy as np
import ml_dtypes

import concourse.bacc as bacc
import concourse.mybir as mybir
import concourse.tile as tile
from concourse.bass_utils import run_bass_kernel_spmd

dt = mybir.dt
BF16 = ml_dtypes.bfloat16
FP8 = ml_dtypes.float8_e4m3
AF = mybir.ActivationFunctionType

N_CORES = 8
TP, DP = 4, 2
B = 1024
HW = 8
C_IN = 8
FN = 16
SEN, INT, OUT = 1024, 2048, 1024
TOT = 4096
CNN_OUT = 3264
CNN_PAD = 3328
NUM_OUT = 1968
NUM_PAD = 2048
LORA_R = 64
LORA_SCALE = 2.0

CSH = TOT // TP              # 1024 W-cols per core
BSH = B // N_CORES           # 128  conv/ip batch shard
BH = B // DP                 # 512  per-core batch (its group's half)
CH = BH // 2                 # 256  AG chunk width
OSH = NUM_PAD // TP          # 512  output-column shard
OBLK = OUT // TP             # 256  O-block row slice per core

KT = TOT // 128              # 32
KT_SEN = SEN // 128          # 8
MT = CSH // 128              # 8 m-tiles of the W shard
CONV_MT = CNN_PAD // 128     # 26
SEN_MT = SEN // 128          # 8
OUT_KT = OUT // 128          # 8


def _build_program(reps: int = 1, use_cc: bool = True):
    nc = bacc.Bacc("TRN2", target_bir_lowering=False, debug=False,
                   enable_asserts=True, num_devices=N_CORES)

    xT_d = nc.dram_tensor("xT", [512, BSH], dt.bfloat16, kind="ExternalInput")
    wbig_d = nc.dram_tensor("wbig", [512, CNN_PAD], dt.bfloat16, kind="ExternalInput")
    cbias_d = nc.dram_tensor("cbias", [CNN_PAD], dt.float32, kind="ExternalInput")
    ipw_d = nc.dram_tensor("ipw", [CNN_PAD, SEN], dt.bfloat16, kind="ExternalInput")
    ipb_d = nc.dram_tensor("ipb", [SEN], dt.float32, kind="ExternalInput")
    at_d = nc.dram_tensor("at", [LORA_R, TOT], dt.bfloat16, kind="ExternalInput")
    bsh_d = nc.dram_tensor("bsh", [LORA_R, CSH], dt.bfloat16, kind="ExternalInput")
    bo_d = nc.dram_tensor("bo", [LORA_R, OBLK], dt.bfloat16, kind="ExternalInput")
    w_d = nc.dram_tensor("w", [TOT, CSH], dt.bfloat16, kind="ExternalInput")
    m2_d = nc.dram_tensor("m2", [TOT, CSH], dt.float8e4, kind="ExternalInput")
    wo_d = nc.dram_tensor("wo", [TOT, OBLK], dt.bfloat16, kind="ExternalInput")
    m2o_d = nc.dram_tensor("m2o", [TOT, OBLK], dt.float8e4, kind="ExternalInput")
    oww_d = nc.dram_tensor("oww", [OUT, OSH], dt.bfloat16, kind="ExternalInput")
    ob_d = nc.dram_tensor("ob", [OSH], dt.float32, kind="ExternalInput")

    outT_d = nc.dram_tensor("outT", [OSH, BH], dt.float32, kind="ExternalOutput")

    RG = [[0, 1, 2, 3], [4, 5, 6, 7]]

    with tile.TileContext(nc) as tc:
        with tc.tile_pool(name="persist", bufs=1) as pers, \
             tc.tile_pool(name="psum", bufs=8, space="PSUM") as psp, \
             tc.tile_pool(name="stream", bufs=2) as stp, \
             tc.tile_pool(name="wbigp", bufs=4) as wbp, \
             tc.tile_pool(name="dramb", bufs=2, space="DRAM") as drb, \
             tc.tile_pool(name="drag", bufs=3, space="DRAM") as drg:

            state_sb = pers.tile([128, KT, BH], dt.bfloat16, tag="state_sb")
            weff_sb = pers.tile([128, KT, CSH], dt.bfloat16, tag="weff_sb")
            weffo_sb = pers.tile([128, KT, OBLK], dt.bfloat16, tag="weffo_sb")
            featT_sb = pers.tile([128, CONV_MT, BSH], dt.bfloat16, tag="featT_sb")
            xT_sb = pers.tile([128, 4, BSH], dt.bfloat16, tag="xT_sb")
            cbias_sb = pers.tile([128, CONV_MT], dt.float32, tag="cbias_sb")
            ipb_sb = pers.tile([128, SEN_MT], dt.float32, tag="ipb_sb")
            ob_sb = pers.tile([128, OSH // 128], dt.float32, tag="ob_sb")
            oww_sb = pers.tile([128, OUT_KT, OSH], dt.bfloat16, tag="oww_sb")

            nc.sync.dma_start(out=xT_sb[:, :, :],
                              in_=xT_d.rearrange("(k p) b -> p k b", p=128))
            nc.sync.dma_start(out=cbias_sb[:], in_=cbias_d.rearrange("(m p) -> p m", p=128))
            nc.sync.dma_start(out=ipb_sb[:], in_=ipb_d.rearrange("(m p) -> p m", p=128))
            nc.sync.dma_start(out=ob_sb[:], in_=ob_d.rearrange("(m p) -> p m", p=128))
            nc.sync.dma_start(out=oww_sb[:, :, :],
                              in_=oww_d.rearrange("(k p) o -> p k o", p=128))

            for rep in range(reps):
                # ---- conv bank ----
                wbig_t = []
                for k in range(4):
                    t = wbp.tile([128, CNN_PAD], dt.bfloat16, tag="wbig")
                    nc.sync.dma_start(out=t[:], in_=wbig_d[k * 128:(k + 1) * 128, :])
                    wbig_t.append(t)
                for m in range(CONV_MT):
                    c_ps = psp.tile([128, BSH], dt.float32, tag="ps")
                    for k in range(4):
                        nc.tensor.matmul(c_ps[:], wbig_t[k][:, m * 128:(m + 1) * 128],
                                         xT_sb[:, k, :], start=(k == 0), stop=(k == 3))
                    nc.scalar.activation(featT_sb[:, m, :], c_ps[:], AF.Relu,
                                         bias=cbias_sb[:, m:m + 1])

                # ---- input proj -> state0 (own conv batch shard) ----
                e_sb = stp.tile([128, SEN_MT, BSH], dt.bfloat16, tag="e_sb", bufs=1)
                ip_ps = [psp.tile([128, BSH], dt.float32, tag="ps", name=f"ip_ps{_m}")
                         for _m in range(SEN_MT)]
                for k in range(CONV_MT):
                    ipw_t = stp.tile([128, SEN], dt.bfloat16, tag="ipw", bufs=4)
                    nc.sync.dma_start(out=ipw_t[:], in_=ipw_d[k * 128:(k + 1) * 128, :])
                    for m in range(SEN_MT):
                        nc.tensor.matmul(ip_ps[m][:], ipw_t[:, m * 128:(m + 1) * 128],
                                         featT_sb[:, k, :], start=(k == 0),
                                         stop=(k == CONV_MT - 1))
                for m in range(SEN_MT):
                    nc.vector.tensor_scalar(e_sb[:, m, :], ip_ps[m][:],
                                            ipb_sb[:, m:m + 1], 0.0,
                                            op0=mybir.AluOpType.add,
                                            op1=mybir.AluOpType.max)

                # ---- AG#0 within group: gather the half's state0 ----
                e_bnc = drb.tile([128, SEN_MT, BSH], dt.bfloat16, tag="e_bnc")
                nc.gpsimd.dma_start(out=e_bnc[:, :, :], in_=e_sb[:, :, :])
                ag0 = drg.tile([TP, 128, SEN_MT, BSH], dt.bfloat16, tag="ag0")
                if use_cc:
                    nc.gpsimd.collective_compute(
                        "AllGather", mybir.AluOpType.bypass, replica_groups=RG,
                        ins=[e_bnc.opt()], outs=[ag0.opt()])
                else:
                    nc.sync.dma_start(out=ag0[0], in_=e_bnc[:, :, :])
                for r in range(TP):
                    nc.gpsimd.dma_start(
                        out=state_sb[:, 0:KT_SEN, r * BSH:(r + 1) * BSH],
                        in_=ag0[r])

                if rep == 0:
                    # ---- build W1 shard on device ----
                    with tc.tile_pool(name="wbuild", bufs=2) as wbd:
                        b_sb = wbd.tile([LORA_R, CSH], dt.bfloat16, tag="b_sb", bufs=1)
                        bo_sb = wbd.tile([LORA_R, OBLK], dt.bfloat16, tag="bo_sb", bufs=1)
                        nc.sync.dma_start(out=b_sb[:], in_=bsh_d[:])
                        nc.sync.dma_start(out=bo_sb[:], in_=bo_d[:])
                        for aj in range(4):   # stream A.T in 4 column chunks
                            at_t = wbd.tile([LORA_R, 1024], dt.bfloat16, tag="at_t")
                            nc.sync.dma_start(out=at_t[:],
                                              in_=at_d[:, aj * 1024:(aj + 1) * 1024])
                            for kk in range(8):
                                k = aj * 8 + kk
                                l_ps = [psp.tile([128, 512], dt.float32, tag="ps",
                                                 name=f"l_ps{k}_{j}") for j in range(2)]
                                for j in range(2):
                                    nc.tensor.matmul(
                                        l_ps[j][:], at_t[:, kk * 128:(kk + 1) * 128],
                                        b_sb[:, j * 512:(j + 1) * 512],
                                        start=True, stop=True)
                                lo_ps = psp.tile([128, OBLK], dt.float32, tag="ps")
                                nc.tensor.matmul(lo_ps[:], at_t[:, kk * 128:(kk + 1) * 128],
                                                 bo_sb[:], start=True, stop=True)
                                w_t = wbd.tile([128, CSH], dt.bfloat16, tag="w_t")
                                nc.sync.dma_start(out=w_t[:], in_=w_d[k * 128:(k + 1) * 128, :])
                                m2_t = wbd.tile([128, CSH], dt.float8e4, tag="m2_t")
                                nc.sync.dma_start(out=m2_t[:], in_=m2_d[k * 128:(k + 1) * 128, :])
                                wo_t = wbd.tile([128, OBLK], dt.bfloat16, tag="wo_t")
                                nc.sync.dma_start(out=wo_t[:], in_=wo_d[k * 128:(k + 1) * 128, :])
                                m2o_t = wbd.tile([128, OBLK], dt.float8e4, tag="m2o_t")
                                nc.sync.dma_start(out=m2o_t[:],
                                                  in_=m2o_d[k * 128:(k + 1) * 128, :])
                                for j in range(2):
                                    sl = slice(j * 512, (j + 1) * 512)
                                    nc.vector.tensor_tensor(
                                        weff_sb[:, k, sl], l_ps[j][:], m2_t[:, sl],
                                        op=mybir.AluOpType.mult)
                                    nc.vector.tensor_tensor(
                                        weff_sb[:, k, sl], weff_sb[:, k, sl], w_t[:, sl],
                                        op=mybir.AluOpType.add)
                                nc.vector.tensor_tensor(weffo_sb[:, k, :], lo_ps[:],
                                                        m2o_t[:], op=mybir.AluOpType.mult)
                                nc.vector.tensor_tensor(weffo_sb[:, k, :], weffo_sb[:, k, :],
                                                        wo_t[:], op=mybir.AluOpType.add)

                # ---- recurrence t1..t3 ----
                for t in (1, 2, 3):
                    nk = KT_SEN if t == 1 else KT
                    for ch in (0, 1):
                        s_wire = stp.tile([128, MT, CH], dt.bfloat16, tag="s_wire",
                                          bufs=2)
                        for m in range(MT):
                            r_ps = psp.tile([128, CH], dt.float32, tag="ps")
                            for k in range(nk):
                                nc.tensor.matmul(
                                    r_ps[:], weff_sb[:, k, m * 128:(m + 1) * 128],
                                    state_sb[:, k, ch * CH:(ch + 1) * CH],
                                    start=(k == 0), stop=(k == nk - 1))
                            nc.vector.tensor_scalar_max(s_wire[:, m, :], r_ps[:], 0.0)
                        s_bnc = drb.tile([128, MT, CH], dt.bfloat16, tag="s_bnc",
                                         bufs=4)
                        nc.gpsimd.dma_start(out=s_bnc[:, :, :], in_=s_wire[:, :, :])
                        ag_st = drg.tile([TP, 128, MT, CH], dt.bfloat16, tag="ag_st",
                                         bufs=6)
                        if use_cc:
                            nc.gpsimd.collective_compute(
                                "AllGather", mybir.AluOpType.bypass, replica_groups=RG,
                                ins=[s_bnc.opt()], outs=[ag_st.opt()])
                        else:
                            nc.sync.dma_start(out=ag_st[0], in_=s_bnc[:, :, :])
                        for r in range(TP):
                            nc.sync.dma_start(
                                out=state_sb[:, r * MT:(r + 1) * MT,
                                             ch * CH:(ch + 1) * CH],
                                in_=ag_st[r])

                # ---- t4: O-block slice [OBLK rows, BH] ----
                o_wire = stp.tile([128, OBLK // 128, BH], dt.bfloat16, tag="o_wire",
                                  bufs=1)
                for ch in (0, 1):
                    for m in range(OBLK // 128):
                        r_ps = psp.tile([128, CH], dt.float32, tag="ps")
                        for k in range(KT):
                            nc.tensor.matmul(r_ps[:],
                                             weffo_sb[:, k, m * 128:(m + 1) * 128],
                                             state_sb[:, k, ch * CH:(ch + 1) * CH],
                                             start=(k == 0), stop=(k == KT - 1))
                        nc.vector.tensor_scalar_max(
                            o_wire[:, m, ch * CH:(ch + 1) * CH], r_ps[:], 0.0)
                o_bnc = drb.tile([128, OBLK // 128, BH], dt.bfloat16, tag="o_bnc")
                nc.gpsimd.dma_start(out=o_bnc[:, :, :], in_=o_wire[:, :, :])
                ag4 = drg.tile([TP, 128, OBLK // 128, BH], dt.bfloat16, tag="ag4")
                if use_cc:
                    nc.gpsimd.collective_compute(
                        "AllGather", mybir.AluOpType.bypass, replica_groups=RG,
                        ins=[o_bnc.opt()], outs=[ag4.opt()])
                else:
                    nc.sync.dma_start(out=ag4[0], in_=o_bnc[:, :, :])
                for r in range(TP):
                    nc.sync.dma_start(
                        out=state_sb[:, KT - OUT_KT + r * 2:KT - OUT_KT + r * 2 + 2, :],
                        in_=ag4[r])

                # ---- output projection ----
                for m in range(OSH // 128):
                    p_ps = psp.tile([128, BH], dt.float32, tag="ps")
                    for k in range(OUT_KT):
                        nc.tensor.matmul(
                            p_ps[:], oww_sb[:, k, m * 128:(m + 1) * 128],
                            state_sb[:, KT - OUT_KT + k, :],
                            start=(k == 0), stop=(k == OUT_KT - 1))
                    o_m = stp.tile([128, BH], dt.float32, tag="o_m", bufs=2)
                    nc.vector.tensor_scalar_add(o_m[:], p_ps[:], ob_sb[:, m:m + 1])
                    nc.sync.dma_start(out=outT_d[m * 128:(m + 1) * 128, :], in_=o_m[:])

    nc.compile()
    return nc


_PROGRAM_CACHE: dict = {}


def get_program(reps: int = 1, use_cc: bool = True):
    key = (reps, use_cc)
    if key not in _PROGRAM_CACHE:
        _PROGRAM_CACHE[key] = _build_program(reps, use_cc)
    return _PROGRAM_CACHE[key]


def _assemble_wbig(inputs):
    wbig = np.zeros((512, CNN_PAD), np.float32)
    cbias = np.zeros(CNN_PAD, np.float32)
    off = 0
    for k in range(1, 9):
        o = HW - k + 1
        w = np.asarray(inputs[f"conv_w{k}"], np.float32)
        cb = np.asarray(inputs["conv_b"], np.float32)[k - 1]
        py = np.arange(o)[:, None, None]
        px = np.arange(o)[None, :, None]
        cc = np.arange(C_IN)[None, None, :]
        ncol = np.arange(FN)[:, None, None]
        cols = off + ncol * o * o + py[None, :, :, 0] * o + px[None, :, :, 0]
        for dy in range(k):
            for dx in range(k):
                rows = (py + dy) * 64 + (px + dx) * 8 + cc
                wbig[rows[None, :, :, :], cols[:, :, :, None]] = \
                    w[:, :, dy, dx][:, None, None, :]
        cbias[off + np.arange(FN * o * o)] = np.repeat(cb, o * o)
        off += FN * o * o
    return wbig, cbias


def _prep_inputs(inputs):
    x = np.asarray(inputs["x"], np.float32)
    W = np.asarray(inputs["W"], np.float32)
    lora_A = np.asarray(inputs["lora_A"], np.float32)
    lora_B = np.asarray(inputs["lora_B"], np.float32)
    ip_w = np.asarray(inputs["ip_w"], np.float32)
    ip_b = np.asarray(inputs["ip_b"], np.float32)
    out_w = np.asarray(inputs["out_w"], np.float32)
    out_b = np.asarray(inputs["out_b"], np.float32)

    wbig, cbias = _assemble_wbig(inputs)
    ipw_pad = np.zeros((CNN_PAD, SEN), np.float32)
    ipw_pad[:CNN_OUT] = ip_w
    oww_pad = np.zeros((OUT, NUM_PAD), np.float32)
    oww_pad[:, :NUM_OUT] = out_w
    ob_pad = np.zeros(NUM_PAD, np.float32)
    ob_pad[:NUM_OUT] = out_b

    at = np.ascontiguousarray(lora_A.T)
    mask2 = (W != 0).astype(np.float32) * LORA_SCALE
    eye = np.eye(TOT, dtype=np.float32)

    def bf(a):
        return np.ascontiguousarray(a).astype(BF16)

    shared = {
        "wbig": bf(wbig), "cbias": np.ascontiguousarray(cbias),
        "ipw": bf(ipw_pad), "ipb": np.ascontiguousarray(ip_b),
        "at": bf(at),
    }
    in_maps = []
    for c in range(N_CORES):
        s = c % TP
        cs = slice(s * CSH, (s + 1) * CSH)
        osl = slice(SEN + INT + s * OBLK, SEN + INT + (s + 1) * OBLK)
        xs = x[c * BSH:(c + 1) * BSH].reshape(BSH, 512).T
        m = dict(shared)
        m["xT"] = bf(xs)
        m["bsh"] = bf(lora_B[:, cs])
        m["bo"] = bf(lora_B[:, osl])
        m["w"] = bf(W[:, cs] + eye[:, cs])
        m["m2"] = np.ascontiguousarray(mask2[:, cs]).astype(FP8)
        m["wo"] = bf(W[:, osl] + eye[:, osl])
        m["m2o"] = np.ascontiguousarray(mask2[:, osl]).astype(FP8)
        m["oww"] = bf(oww_pad[:, s * OSH:(s + 1) * OSH])
        m["ob"] = np.ascontiguousarray(ob_pad[s * OSH:(s + 1) * OSH])
        in_maps.append(m)
    return in_maps


def run_on_hw(in_maps, reps: int = 1):
    nc = get_program(reps)
    return run_bass_kernel_spmd(nc, in_maps, list(range(N_CORES)), trace=False)


def kernel(**inputs) -> np.ndarray:
    in_maps = _prep_inputs(inputs)
    res = run_on_hw(in_maps, reps=1)
    outT = np.zeros((NUM_PAD, B), np.float32)
    for c in range(N_CORES):
        g, s = c // TP, c % TP
        outT[s * OSH:(s + 1) * OSH, g * BH:(g + 1) * BH] = \
            np.asarray(res.results[c]["outT"], np.float32)
    return np.ascontiguousarray(outT[:NUM_OUT].T)



# revision 5
# speedup vs baseline: 3.1929x; 3.1929x over previous
"""Trainium2 Bass kernel for nn_BasicCNN (conv bank + LoRA-masked recurrent net).

Pure data-parallel over batch (128 rows/core on 8 cores), ZERO collectives —
under the conservative collective cost model (15us + bytes/40GBps) any
TP-style per-timestep AllGather dwarfs the actual compute, so each core runs
the full network on its batch shard instead:

 - W1 = W + 2*(A@B)*mask + I is built on HOST (fp32) and shipped bf16.
   The +I fold implements the residual; rows 0:KT_RES*128 stay SBUF-resident,
   the rest is streamed per timestep as [128, 1024] column-chunks (one chunk
   per (m-group, k-slab)) on the SP/Pool DMA queues, double-buffered through
   an 8-slot ring so DMA hides under the PE matmuls.
 - State kept transposed k-major ([state_dim, batch] in 32 slabs of
   [128, 128]) so W tiles are the stationary operand and the matmul output
   [m-part, batch] is directly the next state slab — no transposes ever.
 - PSUM allows only 8 bank-aligned accumulators, so each timestep runs 4
   m-groups x 8 banks; relu drains alternate DVE/Act so banks free fast.
 - t1 contracts only the sensory block (state1 is zero past SEN);
   t4 computes only the O block; conv bank = one dense [512, 3328] matmul
   vs host-assembled scatter of the conv kernels; out projection streamed.
"""
import sys

for _p in ("/opt/trn_rl_repo", "/root/.axon_site/_ro/trn_rl_repo"):
    if _p not in sys.path:
        sys.path.append(_p)

import numpy as np
import ml_dtypes

import concourse.bacc as bacc
import concourse.mybir as mybir
import concourse.tile as tile
from concourse.bass_utils import run_bass_kernel_spmd

dt = mybir.dt
BF16 = ml_dtypes.bfloat16
AF = mybir.ActivationFunctionType

N_CORES = 8
B = 1024
HW = 8
C_IN = 8
FN = 16
SEN, INT, OUT = 1024, 2048, 1024
TOT = 4096
CNN_OUT = 3264
CNN_PAD = 3328
NUM_OUT = 1968
NUM_PAD = 2048
LORA_SCALE = 2.0

BSH = B // N_CORES           # 128 batch rows per core
KT = TOT // 128              # 32 k-slabs of state/W
KT_RES = 13                  # W1 k-slabs resident in SBUF
KT_STR = KT - KT_RES         # 19 streamed per full timestep
NG = 4                       # m-groups per timestep
GM = 8                       # m-tiles per group (= PSUM banks)
CONV_MT = CNN_PAD // 128     # 26
SEN_KT = SEN // 128          # 8
OUT_KT = OUT // 128          # 8 (O-block k-slabs for out proj)
OMT = NUM_PAD // 128         # 16


def _build_program(reps: int = 1, use_cc: bool = True):
    nc = bacc.Bacc("TRN2", target_bir_lowering=False, debug=False,
                   enable_asserts=True, num_devices=N_CORES)

    xT_d = nc.dram_tensor("xT", [512, BSH], dt.bfloat16, kind="ExternalInput")
    wbig_d = nc.dram_tensor("wbig", [512, CNN_PAD], dt.bfloat16, kind="ExternalInput")
    cbias_d = nc.dram_tensor("cbias", [CNN_PAD], dt.float32, kind="ExternalInput")
    ipw_d = nc.dram_tensor("ipw", [CNN_PAD, SEN], dt.bfloat16, kind="ExternalInput")
    ipb_d = nc.dram_tensor("ipb", [SEN], dt.float32, kind="ExternalInput")
    w1r_d = nc.dram_tensor("w1r", [KT_RES * 128, TOT], dt.bfloat16, kind="ExternalInput")
    w1s_d = nc.dram_tensor("w1s", [KT_STR * 128, TOT], dt.bfloat16, kind="ExternalInput")
    oww_d = nc.dram_tensor("oww", [OUT, NUM_PAD], dt.bfloat16, kind="ExternalInput")
    ob_d = nc.dram_tensor("ob", [NUM_PAD], dt.float32, kind="ExternalInput")

    outT_d = nc.dram_tensor("outT", [NUM_PAD, BSH], dt.float32, kind="ExternalOutput")

    with tile.TileContext(nc) as tc:
        with tc.tile_pool(name="pers", bufs=1) as pers, \
             tc.tile_pool(name="psum", bufs=8, space="PSUM") as psp, \
             tc.tile_pool(name="wbigp", bufs=4) as wbp, \
             tc.tile_pool(name="ipwp", bufs=4) as ipp, \
             tc.tile_pool(name="wchk", bufs=8) as wcp, \
             tc.tile_pool(name="owwp", bufs=4) as owp, \
             tc.tile_pool(name="outp", bufs=1) as otp:

            wres = pers.tile([128, KT_RES, TOT], dt.bfloat16, tag="wres")
            st_a = pers.tile([128, KT, BSH], dt.bfloat16, tag="st_a")
            st_b = pers.tile([128, KT, BSH], dt.bfloat16, tag="st_b")
            ostate = pers.tile([128, OUT_KT, BSH], dt.bfloat16, tag="ostate")
            featT = pers.tile([128, CONV_MT, BSH], dt.bfloat16, tag="featT")
            xT_sb = pers.tile([128, 4, BSH], dt.bfloat16, tag="xT_sb")
            cbias_sb = pers.tile([128, CONV_MT], dt.float32, tag="cbias_sb")
            ipb_sb = pers.tile([128, SEN_KT], dt.float32, tag="ipb_sb")
            ob_sb = pers.tile([128, OMT], dt.float32, tag="ob_sb")

            nc.gpsimd.dma_start(out=xT_sb[:, :, :],
                                in_=xT_d.rearrange("(k p) b -> p k b", p=128))
            nc.gpsimd.dma_start(out=cbias_sb[:], in_=cbias_d.rearrange("(m p) -> p m", p=128))
            nc.gpsimd.dma_start(out=ipb_sb[:], in_=ipb_d.rearrange("(m p) -> p m", p=128))
            nc.gpsimd.dma_start(out=ob_sb[:], in_=ob_d.rearrange("(m p) -> p m", p=128))

            for rep in range(reps):
                # ---- conv bank: one dense matmul vs scattered conv kernels ----
                wbig_t = []
                for kk in range(4):
                    t = wbp.tile([128, CNN_PAD], dt.bfloat16, tag="wbig")
                    nc.scalar.dma_start(out=t[:], in_=wbig_d[kk * 128:(kk + 1) * 128, :])
                    wbig_t.append(t)
                for m in range(CONV_MT):
                    ps = psp.tile([128, BSH], dt.float32, tag="ps", name=f"cv{rep}_{m}")
                    for kk in range(4):
                        nc.tensor.matmul(ps[:], wbig_t[kk][:, m * 128:(m + 1) * 128],
                                         xT_sb[:, kk, :], start=(kk == 0), stop=(kk == 3))
                    nc.scalar.activation(featT[:, m, :], ps[:], AF.Relu,
                                         bias=cbias_sb[:, m:m + 1])

                # ---- input proj -> state1 (k-outer over 26 ipw slabs) ----
                ip_ps = [psp.tile([128, BSH], dt.float32, tag="ps", name=f"ip{rep}_{m}")
                         for m in range(SEN_KT)]
                ipw_q = [nc.scalar, nc.sync]
                for k in range(CONV_MT):
                    t = ipp.tile([128, SEN], dt.bfloat16, tag="ipw")
                    ipw_q[k % 2].dma_start(out=t[:], in_=ipw_d[k * 128:(k + 1) * 128, :])
                    for m in range(SEN_KT):
                        nc.tensor.matmul(ip_ps[m][:], t[:, m * 128:(m + 1) * 128],
                                         featT[:, k, :], start=(k == 0),
                                         stop=(k == CONV_MT - 1))
                for m in range(SEN_KT):
                    nc.scalar.activation(st_a[:, m, :], ip_ps[m][:], AF.Relu,
                                         bias=ipb_sb[:, m:m + 1])

                if rep == 0:
                    # resident W1 slabs on SP/Pool, queued behind the ip
                    # weights so the startup-critical loads go first
                    res_q = [nc.sync, nc.gpsimd]
                    for k in range(KT_RES):
                        res_q[k % 2].dma_start(out=wres[:, k, :],
                                               in_=w1r_d[k * 128:(k + 1) * 128, :])

                # ---- recurrence t1..t3 (t1 contracts only the SEN block) ----
                cur, nxt = st_a, st_b
                for t in (1, 2, 3):
                    nk = SEN_KT if t == 1 else KT
                    qi = 0
                    for mg in range(NG):
                        ps = [psp.tile([128, BSH], dt.float32, tag="ps",
                                       name=f"t{t}r{rep}g{mg}_{i}") for i in range(GM)]
                        chunk = None
                        for k in range(nk):
                            if k >= KT_RES:
                                chunk = wcp.tile([128, 1024], dt.bfloat16, tag="wchk")
                                q = (nc.sync, nc.gpsimd)[qi % 2]
                                qi += 1
                                q.dma_start(out=chunk[:],
                                            in_=w1s_d[(k - KT_RES) * 128:(k - KT_RES + 1) * 128,
                                                      mg * 1024:(mg + 1) * 1024])
                            for m8 in range(GM):
                                m = mg * GM + m8
                                if k < KT_RES:
                                    wap = wres[:, k, m * 128:(m + 1) * 128]
                                else:
                                    wap = chunk[:, m8 * 128:(m8 + 1) * 128]
                                nc.tensor.matmul(ps[m8][:], wap, cur[:, k, :],
                                                 start=(k == 0), stop=(k == nk - 1))
                        for m8 in range(GM):
                            m = mg * GM + m8
                            if m8 % 2 == 0:
                                nc.vector.tensor_scalar_max(nxt[:, m, :], ps[m8][:], 0.0)
                            else:
                                nc.scalar.activation(nxt[:, m, :], ps[m8][:], AF.Relu)
                    cur, nxt = nxt, cur

                # ---- t4: only the O block (m-group 3) ----
                t4_ps = [psp.tile([128, BSH], dt.float32, tag="ps",
                                  name=f"t4r{rep}_{i}") for i in range(GM)]
                qi = 0
                chunk = None
                for k in range(KT):
                    if k >= KT_RES:
                        chunk = wcp.tile([128, 1024], dt.bfloat16, tag="wchk")
                        q = (nc.sync, nc.gpsimd)[qi % 2]
                        qi += 1
                        q.dma_start(out=chunk[:],
                                    in_=w1s_d[(k - KT_RES) * 128:(k - KT_RES + 1) * 128,
                                              3 * 1024:4 * 1024])
                    for m8 in range(GM):
                        m = 3 * GM + m8
                        if k < KT_RES:
                            wap = wres[:, k, m * 128:(m + 1) * 128]
                        else:
                            wap = chunk[:, m8 * 128:(m8 + 1) * 128]
                        nc.tensor.matmul(t4_ps[m8][:], wap, cur[:, k, :],
                                         start=(k == 0), stop=(k == KT - 1))
                for m8 in range(GM):
                    if m8 % 2 == 0:
                        nc.vector.tensor_scalar_max(ostate[:, m8, :], t4_ps[m8][:], 0.0)
                    else:
                        nc.scalar.activation(ostate[:, m8, :], t4_ps[m8][:], AF.Relu)

                # ---- output projection (2 groups x 8 banks, oww streamed) ----
                ostg = otp.tile([128, OMT, BSH], dt.float32, tag="ostg")
                qi = 0
                for jg in range(2):
                    op_ps = [psp.tile([128, BSH], dt.float32, tag="ps",
                                      name=f"op{rep}g{jg}_{i}") for i in range(GM)]
                    for k in range(OUT_KT):
                        ch = owp.tile([128, 1024], dt.bfloat16, tag="oww")
                        q = (nc.sync, nc.gpsimd)[qi % 2]
                        qi += 1
                        q.dma_start(out=ch[:],
                                    in_=oww_d[k * 128:(k + 1) * 128,
                                              jg * 1024:(jg + 1) * 1024])
                        for m8 in range(GM):
                            nc.tensor.matmul(op_ps[m8][:], ch[:, m8 * 128:(m8 + 1) * 128],
                                             ostate[:, k, :], start=(k == 0),
                                             stop=(k == OUT_KT - 1))
                    for m8 in range(GM):
                        jm = jg * GM + m8
                        nc.vector.tensor_scalar_add(ostg[:, jm, :], op_ps[m8][:],
                                                    ob_sb[:, jm:jm + 1])
                nc.gpsimd.dma_start(out=outT_d.rearrange("(m p) b -> p m b", p=128),
                                    in_=ostg[:, :, :])

    nc.compile()
    return nc


_PROGRAM_CACHE: dict = {}


def get_program(reps: int = 1, use_cc: bool = True):
    key = (reps, use_cc)
    if key not in _PROGRAM_CACHE:
        _PROGRAM_CACHE[key] = _build_program(reps, use_cc)
    return _PROGRAM_CACHE[key]


def _assemble_wbig(inputs):
    wbig = np.zeros((512, CNN_PAD), np.float32)
    cbias = np.zeros(CNN_PAD, np.float32)
    off = 0
    for k in range(1, 9):
        o = HW - k + 1
        w = np.asarray(inputs[f"conv_w{k}"], np.float32)
        cb = np.asarray(inputs["conv_b"], np.float32)[k - 1]
        py = np.arange(o)[:, None, None]
        px = np.arange(o)[None, :, None]
        cc = np.arange(C_IN)[None, None, :]
        ncol = np.arange(FN)[:, None, None]
        cols = off + ncol * o * o + py[None, :, :, 0] * o + px[None, :, :, 0]
        for dy in range(k):
            for dx in range(k):
                rows = (py + dy) * 64 + (px + dx) * 8 + cc
                wbig[rows[None, :, :, :], cols[:, :, :, None]] = \
                    w[:, :, dy, dx][:, None, None, :]
        cbias[off + np.arange(FN * o * o)] = np.repeat(cb, o * o)
        off += FN * o * o
    return wbig, cbias


def _prep_inputs(inputs):
    x = np.asarray(inputs["x"], np.float32)
    W = np.asarray(inputs["W"], np.float32)
    lora_A = np.asarray(inputs["lora_A"], np.float32)
    lora_B = np.asarray(inputs["lora_B"], np.float32)
    ip_w = np.asarray(inputs["ip_w"], np.float32)
    ip_b = np.asarray(inputs["ip_b"], np.float32)
    out_w = np.asarray(inputs["out_w"], np.float32)
    out_b = np.asarray(inputs["out_b"], np.float32)

    wbig, cbias = _assemble_wbig(inputs)
    ipw_pad = np.zeros((CNN_PAD, SEN), np.float32)
    ipw_pad[:CNN_OUT] = ip_w
    oww_pad = np.zeros((OUT, NUM_PAD), np.float32)
    oww_pad[:, :NUM_OUT] = out_w
    ob_pad = np.zeros(NUM_PAD, np.float32)
    ob_pad[:NUM_OUT] = out_b

    mask = (W != 0).astype(np.float32)
    W1 = W + LORA_SCALE * (lora_A @ lora_B) * mask + np.eye(TOT, dtype=np.float32)

    def bf(a):
        return np.ascontiguousarray(a).astype(BF16)

    shared = {
        "wbig": bf(wbig), "cbias": np.ascontiguousarray(cbias),
        "ipw": bf(ipw_pad), "ipb": np.ascontiguousarray(ip_b),
        "w1r": bf(W1[:KT_RES * 128]), "w1s": bf(W1[KT_RES * 128:]),
        "oww": bf(oww_pad), "ob": np.ascontiguousarray(ob_pad),
    }
    in_maps = []
    for c in range(N_CORES):
        m = dict(shared)
        m["xT"] = bf(x[c * BSH:(c + 1) * BSH].reshape(BSH, 512).T)
        in_maps.append(m)
    return in_maps


def run_on_hw(in_maps, reps: int = 1):
    nc = get_program(reps)
    return run_bass_kernel_spmd(nc, in_maps, list(range(N_CORES)), trace=False)


def kernel(**inputs) -> np.ndarray:
    in_maps = _prep_inputs(inputs)
    res = run_on_hw(in_maps, reps=1)
    out = np.zeros((B, NUM_OUT), np.float32)
    for c in range(N_CORES):
        out[c * BSH:(c + 1) * BSH, :] = \
            np.asarray(res.results[c]["outT"], np.float32)[:NUM_OUT].T
    return np.ascontiguousarray(out)
